# revision 16
# baseline (speedup 1.0000x reference)
"""BRC layer (bistable recurrent cell) Trainium2 Bass kernel.

Reference computation (per time step t, elementwise over (batch, hidden)):
    c   = sigmoid(xc_t + w_c * h)
    a   = 1 + tanh(xa_t + w_a * h)
    h'  = c * h + (1 - c) * tanh(xh_t + a * h)
with xc/xa/xh = x_t @ U_{c,a,h}^T (+ bias).

Sharding: batch x2 (64/core), hidden x4 (128/core) -> 8 cores.
Per-core compute layout: (128 hidden partitions, 64 batch free).

Tricks:
 - everything is expressed through tanh (4-ULP table):
     a - 1 = tanh(za);  c = 0.5 + 0.5*tanh(zc') with the 0.5 scale of
     zc folded into U_c/b_c/w_c host-side.
   One packed (128,128) ACT computes both tanh(za) and tanh(zc').
 - Projections for a chunk of 8 steps are computed by TensorE into PSUM
   (2 K-chunks of 128 + a K=1 matmul that adds the bias via a ones row).
 - scalar_tensor_tensor fuses (h*w) + proj in one DVE op reading PSUM.
 - GPSIMD (Pool) runs the SBUF-only elementwise ops (m/d/e/h') in
   parallel with DVE, which keeps the PSUM-reading fused ops.
 - Output is written DMA-friendly as (h_local, S, b_local); host gathers.
"""

import numpy as np

import concourse.bass as bass
import concourse.mybir as mybir
from concourse import bacc
from concourse.tile import TileContext
from concourse.bass_utils import run_bass_kernel_spmd

F32 = mybir.dt.float32
F32R = mybir.dt.float32r
SEQ, BATCH, INP, HID = 2048, 128, 256, 512
NB, NH = 2, 4                     # batch shards x hidden shards = 8 cores
BL, HL = BATCH // NB, HID // NH   # 64 batch/core, 128 hidden/core
TC = 8                            # time steps per PSUM chunk
RC = 4                            # PSUM chunks per DMA block
MULT = mybir.AluOpType.mult
ADD = mybir.AluOpType.add


def build_bass(seq_len=SEQ, repeats=1, no_pe=False, no_scan=False):
    nc = bacc.Bacc()
    nblock = seq_len // (TC * RC)
    assert seq_len % (TC * RC) == 0

    xT = nc.dram_tensor("xT", (2, 128, seq_len, BL), F32, kind="ExternalInput")
    W = nc.dram_tensor("W", (3, 2, 128, HL), F32, kind="ExternalInput")
    BIAS = nc.dram_tensor("BIAS", (2, 1, HL), F32, kind="ExternalInput")
    WDIAG = nc.dram_tensor("WDIAG", (HL, 2), F32, kind="ExternalInput")
    H0 = nc.dram_tensor("H0", (HL, BL), F32, kind="ExternalInput")
    ONES = nc.dram_tensor("ONES", (1, TC * BL), F32, kind="ExternalInput")
    OUT = nc.dram_tensor("OUT", (HL, seq_len, BL), F32, kind="ExternalOutput")
    HN = nc.dram_tensor("HN", (HL, BL), F32, kind="ExternalOutput")

    TANH = mybir.ActivationFunctionType.Tanh
    SIG = mybir.ActivationFunctionType.Sigmoid

    with TileContext(nc) as tc:
        with (
            tc.tile_pool(name="const", bufs=1) as cpool,
            tc.tile_pool(name="x", bufs=2) as xpool,
            tc.tile_pool(name="step", bufs=6) as spool,
            tc.tile_pool(name="ring", bufs=2) as rpool,
            tc.tile_pool(name="psum", bufs=2, space=bass.MemorySpace.PSUM) as ppool,
        ):
            wt = cpool.tile([128, 3, 2, HL], F32)
            nc.sync.dma_start(wt[:], W.rearrange("a g k m -> k a g m"))
            bias_t = cpool.tile([1, 2, HL], F32)
            nc.sync.dma_start(bias_t[:], BIAS.rearrange("a p m -> p a m"))
            wd = cpool.tile([HL, 2], F32)
            nc.sync.dma_start(wd[:], WDIAG[:])
            h0 = cpool.tile([HL, BL], F32)
            nc.sync.dma_start(h0[:], H0[:])
            ones = cpool.tile([1, TC * BL], F32)
            nc.sync.dma_start(ones[:], ONES[:])
            dummy = cpool.tile([128, TC, BL], F32)
            nc.vector.memset(dummy[:], 0.1)

            for rep in range(repeats):
                prev_ring = None
                for kb in range(nblock):
                    b0 = kb * TC * RC
                    if not no_pe:
                        xt = xpool.tile([128, 2, RC, TC, BL], F32, tag="xt")
                        for g in range(2):
                            nc.sync.dma_start(
                                xt[:, g], xT[g, :, b0 : b0 + RC * TC, :].rearrange(
                                    "p (r t) b -> p r t b", t=TC))

                    ring = rpool.tile([HL, RC, TC, BL], F32, tag="ring")
                    for rc in range(RC):
                        if no_pe:
                            pa = pc = ph = dummy
                        else:
                            pp = ppool.tile([128, 3, TC, BL], F32, tag="pp")
                            xr = [xt[:, g, rc] for g in range(2)]
                            for j in range(3):
                                for g in range(2):
                                    nc.tensor.matmul(
                                        pp[:, j], wt[:, j, g], xr[g],
                                        start=(g == 0),
                                        stop=(g == 1 and j == 2))
                                if j < 2:
                                    nc.tensor.matmul(pp[:, j], bias_t[:, j],
                                                     ones[:], start=False,
                                                     stop=False)
                            st = spool.tile([128, 3, TC, BL], F32, tag="st")
                            nc.vector.tensor_copy(st[:, :, 0 : TC // 2],
                                                  pp[:, :, 0 : TC // 2])
                            nc.vector.tensor_copy(st[:, :, TC // 2 :],
                                                  pp[:, :, TC // 2 :])
                            pa, pc, ph = st[:, 0], st[:, 1], st[:, 2]
                            if no_scan:
                                nc.gpsimd.tensor_add(
                                    ring[:, rc, 0], st[:, 0, 0], st[:, 1, 0])

                        if not no_scan:
                            for t in range(TC):
                                if prev_ring is None and rc == 0 and t == 0:
                                    hp = h0[:]
                                elif rc == 0 and t == 0:
                                    hp = prev_ring[:, RC - 1, TC - 1]
                                elif t == 0:
                                    hp = ring[:, rc - 1, TC - 1]
                                else:
                                    hp = ring[:, rc, t - 1]
                                z = spool.tile([HL, 2, BL], F32, tag="z")
                                nc.vector.scalar_tensor_tensor(
                                    z[:, 0], hp, wd[:, 0:1], pa[:, t],
                                    op0=MULT, op1=ADD)
                                nc.vector.scalar_tensor_tensor(
                                    z[:, 1], hp, wd[:, 1:2], pc[:, t],
                                    op0=MULT, op1=ADD)
                                sg = spool.tile([HL, 2, BL], F32, tag="sg")
                                nc.scalar.activation(sg[:], z[:], SIG)
                                m = spool.tile([HL, BL], F32, tag="m")
                                nc.vector.tensor_mul(m[:], sg[:, 0], hp)
                                arg = spool.tile([HL, BL], F32, tag="arg")
                                nc.vector.scalar_tensor_tensor(
                                    arg[:], m[:], 2.0, ph[:, t],
                                    op0=MULT, op1=ADD)
                                th = spool.tile([HL, BL], F32, tag="th")
                                nc.scalar.activation(th[:], arg[:], TANH)
                                d = spool.tile([HL, BL], F32, tag="d")
                                nc.gpsimd.tensor_sub(d[:], hp, th[:])
                                e = spool.tile([HL, BL], F32, tag="e")
                                nc.gpsimd.tensor_mul(e[:], sg[:, 1], d[:])
                                nc.gpsimd.tensor_add(ring[:, rc, t], e[:], th[:])

                    nc.gpsimd.dma_start(
                        OUT[:, b0 : b0 + RC * TC, :].rearrange(
                            "p (r t) b -> p r t b", t=TC), ring[:])
                    prev_ring = ring

            nc.sync.dma_start(HN[:], prev_ring[:, RC - 1, TC - 1])
    nc.compile()
    return nc


_NC_CACHE = {}


def _get_nc(seq_len, repeats=1, **kw):
    key = (seq_len, repeats, tuple(sorted(kw.items())))
    if key not in _NC_CACHE:
        _NC_CACHE[key] = build_bass(seq_len, repeats, **kw)
    return _NC_CACHE[key]


def _prep_inputs(x_seq, h, U_c, w_c, b_c, U_a, w_a, b_a, U_h, seq_len):
    xT_half = []
    for bg in range(NB):
        xs = np.ascontiguousarray(
            x_seq[:, bg * BL : (bg + 1) * BL, :].transpose(2, 0, 1))
        xT_half.append(xs.reshape(2, 128, seq_len, BL))
    in_maps = []
    for c in range(NB * NH):
        bg, hg = divmod(c, NH)
        hs = slice(hg * HL, (hg + 1) * HL)
        W = np.stack([
            np.ascontiguousarray((2.0 * U_a[hs]).T).reshape(2, 128, HL),
            np.ascontiguousarray(U_c[hs].T).reshape(2, 128, HL),
            np.ascontiguousarray(U_h[hs].T).reshape(2, 128, HL),
        ]).astype(np.float32)
        BI = np.stack([2.0 * b_a[hs], b_c[hs]]).reshape(2, 1, HL).astype(np.float32)
        WD = np.stack([2.0 * w_a[hs], w_c[hs]], axis=1).astype(np.float32)
        H0 = np.ascontiguousarray(h[bg * BL : (bg + 1) * BL, hs].T).astype(np.float32)
        in_maps.append({
            "xT": xT_half[bg], "W": W, "BIAS": BI, "WDIAG": WD, "H0": H0,
            "ONES": np.ones((1, TC * BL), np.float32),
        })
    return in_maps


def _run(x_seq, h, U_c, w_c, b_c, U_a, w_a, b_a, U_h, seq_len=SEQ, repeats=1,
         **spmd_kw):
    nc = _get_nc(seq_len, repeats)
    in_maps = _prep_inputs(x_seq, h, U_c, w_c, b_c, U_a, w_a, b_a, U_h, seq_len)
    res = run_bass_kernel_spmd(nc, in_maps, core_ids=list(range(8)), **spmd_kw)
    out = np.empty((seq_len, BATCH, HID), np.float32)
    hn = np.empty((BATCH, HID), np.float32)
    for c in range(NB * NH):
        bg, hg = divmod(c, NH)
        bs = slice(bg * BL, (bg + 1) * BL)
        hs = slice(hg * HL, (hg + 1) * HL)
        out[:, bs, hs] = res.results[c]["OUT"].transpose(1, 2, 0)
        hn[bs, hs] = res.results[c]["HN"].T
    return out, hn, res


def kernel(x_seq, h, U_c, w_c, b_c, U_a, w_a, b_a, U_h):
    out, hn, _ = _run(np.asarray(x_seq), np.asarray(h), np.asarray(U_c),
                      np.asarray(w_c), np.asarray(b_c), np.asarray(U_a),
                      np.asarray(w_a), np.asarray(b_a), np.asarray(U_h))
    return out, hn


# revision 18
# speedup vs baseline: 1.8292x; 1.8292x over previous
"""BRC layer (bistable recurrent cell) Trainium2 Bass kernel.

Reference computation (per time step t, elementwise over (batch, hidden)):
    c   = sigmoid(xc_t + w_c * h)
    a   = 1 + tanh(xa_t + w_a * h)
    h'  = c * h + (1 - c) * tanh(xh_t + a * h)
with xc/xa/xh = x_t @ U_{c,a,h}^T (+ bias).

Sharding: batch x2 (64/core), hidden x4 (128/core) -> 8 cores.
Per-core compute layout: (128 hidden partitions, 64 batch free).

Tricks:
 - everything is expressed through tanh (4-ULP table):
     a - 1 = tanh(za);  c = 0.5 + 0.5*tanh(zc') with the 0.5 scale of
     zc folded into U_c/b_c/w_c host-side.
   One packed (128,128) ACT computes both tanh(za) and tanh(zc').
 - Projections for a chunk of 8 steps are computed by TensorE into PSUM
   (2 K-chunks of 128 + a K=1 matmul that adds the bias via a ones row).
 - scalar_tensor_tensor fuses (h*w) + proj in one DVE op reading PSUM.
 - GPSIMD (Pool) runs the SBUF-only elementwise ops (m/d/e/h') in
   parallel with DVE, which keeps the PSUM-reading fused ops.
 - Output is written DMA-friendly as (h_local, S, b_local); host gathers.
"""

import numpy as np

import concourse.bass as bass
import concourse.mybir as mybir
from concourse import bacc
from concourse.tile import TileContext
from concourse.bass_utils import run_bass_kernel_spmd

F32 = mybir.dt.float32
F32R = mybir.dt.float32r
SEQ, BATCH, INP, HID = 2048, 128, 256, 512
NB, NH = 2, 4                     # batch shards x hidden shards = 8 cores
BL, HL = BATCH // NB, HID // NH   # 64 batch/core, 128 hidden/core
TC = 8                            # time steps per PSUM chunk
RC = 4                            # PSUM chunks per DMA block
MULT = mybir.AluOpType.mult
ADD = mybir.AluOpType.add


def build_bass(seq_len=SEQ, repeats=1, no_pe=False, no_scan=False):
    nc = bacc.Bacc()
    nblock = seq_len // (TC * RC)
    assert seq_len % (TC * RC) == 0

    xT = nc.dram_tensor("xT", (2, 128, seq_len, BL), F32, kind="ExternalInput")
    W = nc.dram_tensor("W", (3, 2, 128, HL), F32, kind="ExternalInput")
    BIAS = nc.dram_tensor("BIAS", (2, 1, HL), F32, kind="ExternalInput")
    WDIAG = nc.dram_tensor("WDIAG", (HL, 2), F32, kind="ExternalInput")
    H0 = nc.dram_tensor("H0", (HL, BL), F32, kind="ExternalInput")
    ONES = nc.dram_tensor("ONES", (1, TC * BL), F32, kind="ExternalInput")
    OUT = nc.dram_tensor("OUT", (HL, seq_len, BL), F32, kind="ExternalOutput")
    HN = nc.dram_tensor("HN", (HL, BL), F32, kind="ExternalOutput")

    TANH = mybir.ActivationFunctionType.Tanh
    SIG = mybir.ActivationFunctionType.Sigmoid

    with TileContext(nc) as tc:
        with (
            tc.tile_pool(name="const", bufs=1) as cpool,
            tc.tile_pool(name="x", bufs=2) as xpool,
            tc.tile_pool(name="step", bufs=4) as spool,
            tc.tile_pool(name="ring", bufs=2) as rpool,
            tc.tile_pool(name="psum", bufs=2, space=bass.MemorySpace.PSUM) as ppool,
        ):
            wt = cpool.tile([128, 3, 2, HL], F32)
            nc.sync.dma_start(wt[:], W.rearrange("a g k m -> k a g m"))
            bias_t = cpool.tile([1, 2, HL], F32)
            nc.sync.dma_start(bias_t[:], BIAS.rearrange("a p m -> p a m"))
            wd = cpool.tile([HL, 2], F32)
            nc.sync.dma_start(wd[:], WDIAG[:])
            h0 = cpool.tile([HL, BL], F32)
            nc.sync.dma_start(h0[:], H0[:])
            ones = cpool.tile([1, TC * BL], F32)
            nc.sync.dma_start(ones[:], ONES[:])
            dummy = cpool.tile([128, TC, BL], F32)
            nc.vector.memset(dummy[:], 0.1)

            for rep in range(repeats):
                prev_ring = None
                for kb in range(nblock):
                    b0 = kb * TC * RC
                    if not no_pe:
                        xt = xpool.tile([128, 2, RC, TC, BL], F32, tag="xt")
                        for g in range(2):
                            nc.sync.dma_start(
                                xt[:, g], xT[g, :, b0 : b0 + RC * TC, :].rearrange(
                                    "p (r t) b -> p r t b", t=TC))

                    ring = rpool.tile([HL, RC, TC, BL], F32, tag="ring")
                    for rc in range(RC):
                        if no_pe:
                            pa = pc = ph = dummy
                        else:
                            pp = ppool.tile([128, 3, TC, BL], F32, tag="pp")
                            xr = [xt[:, g, rc] for g in range(2)]
                            for j in range(3):
                                for g in range(2):
                                    nc.tensor.matmul(
                                        pp[:, j], wt[:, j, g], xr[g],
                                        start=(g == 0),
                                        stop=(g == 1 and j == 2))
                                if j < 2:
                                    nc.tensor.matmul(pp[:, j], bias_t[:, j],
                                                     ones[:], start=False,
                                                     stop=False)
                            st = spool.tile([128, 3, TC, BL], F32, tag="st")
                            nc.vector.tensor_copy(st[:, :, 0 : TC // 2],
                                                  pp[:, :, 0 : TC // 2])
                            nc.vector.tensor_copy(st[:, :, TC // 2 :],
                                                  pp[:, :, TC // 2 :])
                            pa, pc, ph = st[:, 0], st[:, 1], st[:, 2]
                            if no_scan:
                                nc.gpsimd.tensor_add(
                                    ring[:, rc, 0], st[:, 0, 0], st[:, 1, 0])

                        if not no_scan:
                            for t in range(TC):
                                if prev_ring is None and rc == 0 and t == 0:
                                    hp = h0[:]
                                elif rc == 0 and t == 0:
                                    hp = prev_ring[:, RC - 1, TC - 1]
                                elif t == 0:
                                    hp = ring[:, rc - 1, TC - 1]
                                else:
                                    hp = ring[:, rc, t - 1]
                                z = spool.tile([HL, 2, BL], F32, tag="z")
                                nc.vector.scalar_tensor_tensor(
                                    z[:, 0], hp, wd[:, 0:1], pa[:, t],
                                    op0=MULT, op1=ADD)
                                nc.vector.scalar_tensor_tensor(
                                    z[:, 1], hp, wd[:, 1:2], pc[:, t],
                                    op0=MULT, op1=ADD)
                                sg = spool.tile([HL, 2, BL], F32, tag="sg")
                                nc.scalar.activation(sg[:], z[:], SIG)
                                m = spool.tile([HL, BL], F32, tag="m")
                                nc.vector.tensor_mul(m[:], sg[:, 0], hp)
                                arg = spool.tile([HL, BL], F32, tag="arg")
                                nc.vector.scalar_tensor_tensor(
                                    arg[:], m[:], 2.0, ph[:, t],
                                    op0=MULT, op1=ADD)
                                th = spool.tile([HL, BL], F32, tag="th")
                                nc.scalar.activation(th[:], arg[:], TANH)
                                d = spool.tile([HL, BL], F32, tag="d")
                                nc.gpsimd.tensor_sub(d[:], hp, th[:])
                                e = spool.tile([HL, BL], F32, tag="e")
                                nc.gpsimd.tensor_mul(e[:], sg[:, 1], d[:])
                                nc.gpsimd.tensor_add(ring[:, rc, t], e[:], th[:])

                    nc.gpsimd.dma_start(
                        OUT[:, b0 : b0 + RC * TC, :].rearrange(
                            "p (r t) b -> p r t b", t=TC), ring[:])
                    prev_ring = ring

            nc.sync.dma_start(HN[:], prev_ring[:, RC - 1, TC - 1])
    nc.compile()
    return nc


_NC_CACHE = {}


def _get_nc(seq_len, repeats=1, **kw):
    key = (seq_len, repeats, tuple(sorted(kw.items())))
    if key not in _NC_CACHE:
        _NC_CACHE[key] = build_bass(seq_len, repeats, **kw)
    return _NC_CACHE[key]


def _prep_inputs(x_seq, h, U_c, w_c, b_c, U_a, w_a, b_a, U_h, seq_len):
    xT_half = []
    for bg in range(NB):
        xs = np.ascontiguousarray(
            x_seq[:, bg * BL : (bg + 1) * BL, :].transpose(2, 0, 1))
        xT_half.append(xs.reshape(2, 128, seq_len, BL))
    in_maps = []
    for c in range(NB * NH):
        bg, hg = divmod(c, NH)
        hs = slice(hg * HL, (hg + 1) * HL)
        W = np.stack([
            np.ascontiguousarray((2.0 * U_a[hs]).T).reshape(2, 128, HL),
            np.ascontiguousarray(U_c[hs].T).reshape(2, 128, HL),
            np.ascontiguousarray(U_h[hs].T).reshape(2, 128, HL),
        ]).astype(np.float32)
        BI = np.stack([2.0 * b_a[hs], b_c[hs]]).reshape(2, 1, HL).astype(np.float32)
        WD = np.stack([2.0 * w_a[hs], w_c[hs]], axis=1).astype(np.float32)
        H0 = np.ascontiguousarray(h[bg * BL : (bg + 1) * BL, hs].T).astype(np.float32)
        in_maps.append({
            "xT": xT_half[bg], "W": W, "BIAS": BI, "WDIAG": WD, "H0": H0,
            "ONES": np.ones((1, TC * BL), np.float32),
        })
    return in_maps


def _run(x_seq, h, U_c, w_c, b_c, U_a, w_a, b_a, U_h, seq_len=SEQ, repeats=1,
         **spmd_kw):
    nc = _get_nc(seq_len, repeats)
    in_maps = _prep_inputs(x_seq, h, U_c, w_c, b_c, U_a, w_a, b_a, U_h, seq_len)
    res = run_bass_kernel_spmd(nc, in_maps, core_ids=list(range(8)), **spmd_kw)
    out = np.empty((seq_len, BATCH, HID), np.float32)
    hn = np.empty((BATCH, HID), np.float32)
    for c in range(NB * NH):
        bg, hg = divmod(c, NH)
        bs = slice(bg * BL, (bg + 1) * BL)
        hs = slice(hg * HL, (hg + 1) * HL)
        out[:, bs, hs] = res.results[c]["OUT"].transpose(1, 2, 0)
        hn[bs, hs] = res.results[c]["HN"].T
    return out, hn, res


def kernel(x_seq, h, U_c, w_c, b_c, U_a, w_a, b_a, U_h):
    out, hn, _ = _run(np.asarray(x_seq), np.asarray(h), np.asarray(U_c),
                      np.asarray(w_c), np.asarray(b_c), np.asarray(U_a),
                      np.asarray(w_a), np.asarray(b_a), np.asarray(U_h))
    return out, hn
